# revision 1
# baseline (speedup 1.0000x reference)
"""Circulant matmul kernel for Trainium2 (8 NeuronCores, SPMD).

Problem: out = input @ K + bias, where K[c, n] = weight[(c - n) mod 4096],
input is [1024, 4096] f32, weight/bias are [4096] f32.

Strategy (tensor-parallel / column-shard, per the sharding hint):
  - Core c computes out[:, 512c:512(c+1)] = X @ K_c + bias_c in fp32 PSUM.
    No collectives; host concatenates the 8 column slices.
  - Mixed precision on the contraction: 8 of 32 contraction chunks run
    as fp8e4 DoubleRow pair-matmuls.  A matmul instruction costs its
    free-dim cycles (216ns at N=512) regardless of dtype, but a
    DoubleRow pair covers TWO chunks per instruction, so the fp8
    chunks halve their instruction count.  Measured rel err ~1.8e-2
    vs the 2e-2 gate caps the fp8 fraction at 8 chunks.
  - Scales keep every fp8 operand out of e4m3's subnormal range while
    all chunks accumulate into one PSUM group: x carries x*4, K
    carries w*256 (exact powers of two in bf16), so PSUM holds
    1024*out and the epilogue multiplies by 2^-10 before adding the
    unscaled f32 bias.

Device kernel structure (per core):
  - xt chunks on the sync HWDGE ring, kc chunks on the scalar HWDGE
    ring; fp8 pairs interleaved among bf16 chunks in both DMA and
    matmul order so PE demand (a pair is 2x cheaper per DMA'd byte)
    never outruns the DMA rings.
  - PE warm-up: matmuls on a scratch tile whose only writer covers a
    disjoint region, so they carry no dependency, issue the moment the
    Tensor engine enters main, and lift the HAM clock gate while the
    first input chunks are still in flight.
  - Phase 1 runs the interleaved chunk list co-major (matches DMA
    arrival); phase 2 finishes each batch tile in turn (bt-major) so
    the rescale + bias + output-DMA epilogues overlap the remaining
    matmuls.
"""

import numpy as np
import ml_dtypes

import concourse.bass as bass
import concourse.mybir as mybir
import concourse.tile as tile
from concourse import bacc
from concourse.bass import ts
from concourse.bass_utils import run_bass_kernel_spmd

N = 4096
BATCH = 1024
NCORES = 8
NSHARD = N // NCORES          # 512 output columns per core
P = 128                       # partitions
CO = N // P                   # 32 contraction chunks
BT = BATCH // P               # 8 batch tiles

FP8_PAIRS = 4                 # leading chunks done as fp8 DoubleRow pairs
CO8 = 2 * FP8_PAIRS           # fp8 chunks
COB = CO - CO8                # bf16 chunks
COB_PH1 = COB - BT            # bf16 chunks processed co-major in phase 1

SX = 4.0                      # x scale (power of 2)
SW = 256.0                    # w scale (power of 2); SX*SW = 1024
INV_S = 2.0 ** -10

N_WARMUP = 8                  # dummy matmuls to lift the HAM clock gate

BF16 = mybir.dt.bfloat16
FP8 = mybir.dt.float8e4
F32 = mybir.dt.float32


def build_nc():
    """Build the per-core Bass program (same program on all cores; data differs)."""
    nc = bacc.Bacc("TRN2", target_bir_lowering=False, debug=False)

    xt8_d = nc.dram_tensor("xt8", [CO8 * P, BATCH], FP8, kind="ExternalInput").ap()
    kc8_d = nc.dram_tensor("kc8", [CO8 * P, NSHARD], FP8, kind="ExternalInput").ap()
    xt_d = nc.dram_tensor("xt", [COB * P, BATCH], BF16, kind="ExternalInput").ap()
    kc_d = nc.dram_tensor("kc", [COB * P, NSHARD], BF16, kind="ExternalInput").ap()
    bias_d = nc.dram_tensor("biasb", [P, NSHARD], F32, kind="ExternalInput").ap()
    out_d = nc.dram_tensor("out", [BATCH, NSHARD], BF16, kind="ExternalOutput").ap()

    xt8_r = xt8_d.rearrange("(co ci) b -> ci co b", ci=P)    # [128, 8, 1024]
    kc8_r = kc8_d.rearrange("(co ci) n -> ci co n", ci=P)    # [128, 8, 512]
    xt_r = xt_d.rearrange("(co ci) b -> ci co b", ci=P)      # [128, 24, 1024]
    kc_r = kc_d.rearrange("(co ci) n -> ci co n", ci=P)      # [128, 24, 512]

    with tile.TileContext(nc) as tc:
        with (
            tc.tile_pool(name="x8pool", bufs=FP8_PAIRS) as x8pool,
            tc.tile_pool(name="k8pool", bufs=FP8_PAIRS) as k8pool,
            tc.tile_pool(name="xpool", bufs=COB) as xpool,
            tc.tile_pool(name="kpool", bufs=COB) as kpool,
            tc.tile_pool(name="cpool", bufs=1) as cpool,
            tc.tile_pool(name="tpool", bufs=2) as tpool,
            tc.tile_pool(name="opool", bufs=4) as opool,
            tc.tile_pool(name="psum", bufs=BT, space="PSUM") as psum_pool,
        ):
            # scratch for PE warm-up. Tile requires *a* writer for the tile,
            # but the warm-up matmuls read a region disjoint from the memset
            # so they carry no dependency and start immediately.
            scratch = cpool.tile([P, NSHARD + P], BF16, tag="scratch")
            nc.vector.memset(scratch[:, 0:1], 0.125)

            # phase-1 consumption order: fp8 pairs interleaved among bf16
            # chunks so PE demand (a pair is ~2x cheaper per DMA'd byte)
            # never outruns the DMA rings.  DMA issue order matches.
            schedule = []
            pair_after = {0: 1, 1: 3, 2: 5, 3: 7}   # pair p after these b items
            next_p = 0
            for co in range(COB_PH1):
                schedule.append(("b", co))
                while next_p < FP8_PAIRS and pair_after[next_p] == co:
                    schedule.append(("p", next_p))
                    next_p += 1

            x8_tiles = [None] * FP8_PAIRS
            k8_tiles = [None] * FP8_PAIRS
            xt_tiles = [None] * COB
            kc_tiles = [None] * COB
            for kind, i in schedule:
                if kind == "b":
                    ktt = kpool.tile([P, NSHARD], BF16, tag="kc")
                    nc.scalar.dma_start(ktt[:], kc_r[:, i, :])
                    kc_tiles[i] = ktt
                    xtt = xpool.tile([P, BATCH], BF16, tag="xt")
                    nc.sync.dma_start(xtt[:], xt_r[:, i, :])
                    xt_tiles[i] = xtt
                else:
                    k8t = k8pool.tile([P, 2, NSHARD], FP8, tag="kc8")
                    nc.scalar.dma_start(k8t[:, 0, :], kc8_r[:, 2 * i, :])
                    nc.scalar.dma_start(k8t[:, 1, :], kc8_r[:, 2 * i + 1, :])
                    k8_tiles[i] = k8t
                    x8t = x8pool.tile([P, 2, BATCH], FP8, tag="xt8")
                    nc.sync.dma_start(x8t[:, 0, :], xt8_r[:, 2 * i, :])
                    nc.sync.dma_start(x8t[:, 1, :], xt8_r[:, 2 * i + 1, :])
                    x8_tiles[i] = x8t
            # phase-2 bf16 chunks after the interleaved block
            for co in range(COB_PH1, COB):
                ktt = kpool.tile([P, NSHARD], BF16, tag="kc")
                nc.scalar.dma_start(ktt[:], kc_r[:, co, :])
                kc_tiles[co] = ktt
                xtt = xpool.tile([P, BATCH], BF16, tag="xt")
                nc.sync.dma_start(xtt[:], xt_r[:, co, :])
                xt_tiles[co] = xtt
            # bias last on the scalar ring: only needed for the epilogues
            bias_sb = cpool.tile([P, NSHARD], F32, tag="bias")
            nc.scalar.dma_start(bias_sb[:], bias_d)

            psum_tiles = [
                psum_pool.tile([P, NSHARD], F32, tag="ps", name=f"ps{bt}")
                for bt in range(BT)
            ]

            # PE warm-up: full-width dummy matmuls reading garbage
            for i in range(N_WARMUP):
                nc.tensor.matmul(
                    psum_tiles[i % BT][:],
                    scratch[:, P : 2 * P],
                    scratch[:, P : P + NSHARD],
                    start=True,
                    stop=True,
                )

            def bf_mm(co, bt, start=False, stop=False):
                nc.tensor.matmul(
                    psum_tiles[bt][:],
                    xt_tiles[co][:, ts(bt, P)],        # lhsT [c=128, b=128]
                    kc_tiles[co][:],                   # rhs  [c=128, n=512]
                    start=start,
                    stop=stop,
                )

            def dr_mm(p, bt):
                nc.tensor.matmul(
                    psum_tiles[bt][:],
                    x8_tiles[p][:, :, ts(bt, P)],      # lhsT [c=128, 2, b=128]
                    k8_tiles[p][:],                    # rhs  [c=128, 2, n=512]
                    start=False,
                    stop=False,
                    perf_mode=mybir.MatmulPerfMode.DoubleRow,
                )

            # phase 1: one DoubleRow MM after every 4 bf16 MMs.  DoubleRow
            # activity reads as (partially) idle to the HAM clock-gate
            # monitor; keeping bf16 density >= 80% everywhere prevents the
            # mid-stream re-throttle oscillation seen with denser bursts.
            n_bf = 0
            n_dr = 0
            for co in range(COB_PH1):
                for bt in range(BT):
                    bf_mm(co, bt, start=(co == 0))
                    n_bf += 1
                    # start inserting once pair-0's DMA has landed (~2 items)
                    if n_bf >= 16 and (n_bf - 16) % 3 == 0 and n_dr < FP8_PAIRS * BT:
                        dr_mm(n_dr // BT, n_dr % BT)
                        n_dr += 1
            assert n_dr == FP8_PAIRS * BT

            # phase 2: finish batch tiles one at a time; epilogue overlaps MMs
            for bt in range(BT):
                for co in range(COB_PH1, COB):
                    nc.tensor.matmul(
                        psum_tiles[bt][:],
                        xt_tiles[co][:, ts(bt, P)],
                        kc_tiles[co][:],
                        start=False,
                        stop=(co == COB - 1),
                    )
                tmp = tpool.tile([P, NSHARD], F32, tag="tmp")
                out_sb = opool.tile([P, NSHARD], BF16, tag="osb")
                if bt < BT - 1:
                    # rescale on the idle ACT engine; bias-add on DVE
                    nc.scalar.activation(
                        tmp[:], psum_tiles[bt][:],
                        mybir.ActivationFunctionType.Copy, scale=INV_S,
                    )
                    nc.vector.tensor_add(out_sb[:], tmp[:], bias_sb[:])
                    nc.sync.dma_start(out_d[ts(bt, P), :], out_sb[:])
                else:
                    # last tile: halve the epilogue and pipeline ACT rescale
                    # against DVE bias-add so only ~one half-epilogue remains
                    # exposed after the final matmul
                    for h in range(2):
                        sl = slice(h * (NSHARD // 2), (h + 1) * (NSHARD // 2))
                        nc.scalar.activation(
                            tmp[:, sl], psum_tiles[bt][:, sl],
                            mybir.ActivationFunctionType.Copy, scale=INV_S,
                        )
                        nc.vector.tensor_add(
                            out_sb[:, sl], tmp[:, sl], bias_sb[:, sl]
                        )
                        # halves on different HWDGE rings: parallel
                        # descriptor generation for the final two DMAs
                        ring = nc.scalar if h == 0 else nc.sync
                        ring.dma_start(out_d[ts(bt, P), sl], out_sb[:, sl])

    nc.compile()
    return nc


def prepare_in_maps(input, weight, bias):
    x = np.asarray(input, dtype=np.float32)
    w = np.asarray(weight, dtype=np.float32)
    b = np.asarray(bias, dtype=np.float32)

    xs = np.ascontiguousarray(x.T) * SX                         # [4096, 1024]
    xt8 = xs[: CO8 * P].astype(ml_dtypes.float8_e4m3fn)
    xtb = xs[CO8 * P :].astype(ml_dtypes.bfloat16)

    c = np.arange(N)
    in_maps = []
    for core in range(NCORES):
        n0 = core * NSHARD
        idx = (c[:, None] - (n0 + np.arange(NSHARD))[None, :]) % N
        ks = w[idx] * SW                                        # [4096, 512]
        kc8 = ks[: CO8 * P].astype(ml_dtypes.float8_e4m3fn)
        kcb = ks[CO8 * P :].astype(ml_dtypes.bfloat16)
        bias_tile = np.ascontiguousarray(
            np.broadcast_to(b[n0 : n0 + NSHARD].astype(np.float32), (P, NSHARD))
        )
        in_maps.append(
            {"xt8": xt8, "kc8": kc8, "xt": xtb, "kc": kcb, "biasb": bias_tile}
        )
    return in_maps


_NC_CACHE = None


def _get_nc():
    global _NC_CACHE
    if _NC_CACHE is None:
        _NC_CACHE = build_nc()
    return _NC_CACHE


def kernel(**inputs):
    nc = _get_nc()
    in_maps = prepare_in_maps(inputs["input"], inputs["weight"], inputs["bias"])
    res = run_bass_kernel_spmd(nc, in_maps, list(range(NCORES)))
    out = np.empty((BATCH, N), dtype=np.float32)
    for core in range(NCORES):
        out[:, core * NSHARD : (core + 1) * NSHARD] = res.results[core]["out"].astype(
            np.float32
        )
    return out



# revision 6
# speedup vs baseline: 1.6589x; 1.6589x over previous
"""Circulant matmul kernel for Trainium2 (8 NeuronCores, SPMD).

Problem: out = input @ K + bias, K[i, k] = weight[(i - k) mod 4096],
input [1024, 4096] f32, weight/bias [4096] f32.

Algorithm — 2-level real CRT splitting of the circulant (exact):
  out = x (circ-conv) v + bias, v[j] = w[(-j) mod n].
  R[X]/(X^4096-1) ~ R[X]/(X^2048-1) x R[X]/(X^2048+1), and the cyclic
  factor splits once more.  Device work becomes three dense GEMMs
    B0: y0 = x_m  @ M0   (nega-circulant 2048, M0 = 1/2 * NC(v_m))
    B1: y1 = x_pm @ M1   (nega-circulant 1024, M1 = 1/4 * NC(v_pm))
    B2: y2 = x_pp @ M2   (circulant      1024, M2 = 1/4 *  C(v_pp))
  i.e. 6.29M weight elems instead of 16.8M (37%), with the split
  butterflies and final recombine done on host (pure adds, exact).

Sharding: 4-way column shard x 2-way batch shard (core = (q, h)).
  Per core: X side [512 rows, 4096], W side 1.57M elems, out [512, 1024].
  No collectives; host recombines.

Precision: bf16 operands, f32 PSUM, except F8_PAIRS*2 chunks of B0 run
as fp8e4 DoubleRow pair-matmuls (error-budgeted; sim predicts ~1.8e-2
rel err at 4 pairs vs the 2e-2 gate).  All chunks share the scale
product sx*sw = 4*256 = 1024 so every block accumulates unscaled-
compatibly; the epilogue multiplies by 2^-10 (exact powers of two).

Device schedule (per core, mirrors the proven baseline idioms):
  - X units on the sync HWDGE ring, W units on the scalar ring; 2-chunk
    units = 1 DMA each; issue order matches matmul consumption order.
  - PE warm-up matmuls on a scratch tile (no deps) lift the HAM clock
    gate while the first tiles are in flight.
  - Phase 1 co-major (chunk-unit at a time across all 4 batch tiles);
    phase 2 finishes each batch tile in turn so the ACT/DVE epilogues
    and output DMAs overlap the remaining matmuls.
  - Epilogues split across engines: psA (B0) rescale on ACT, psB
    (B1|B2) rescale on DVE, output DMAs on the sync ring.
"""

import numpy as np
import ml_dtypes

import concourse.bass as bass
import concourse.mybir as mybir
import concourse.tile as tile
from concourse import bacc
from concourse.bass import ts
from concourse.bass_utils import run_bass_kernel_spmd

N = 4096
BATCH = 1024
NCORES = 8
CQ = 4                    # column shards
BH = 2                    # batch halves
R = BATCH // BH           # 512 rows per core
P = 128
BT = R // P               # 4 batch tiles per core

N0, N1, N2 = 2048, 1024, 1024          # block sizes
C0, C1, C2 = N0 // P, N1 // P, N2 // P  # chunks: 16, 8, 8
K0, K1, K2 = N0 // CQ, N1 // CQ, N2 // CQ  # cols/core: 512, 256, 256
NOUT = K0 + K1 + K2       # 1024 out cols per core

F8_PAIRS = 4              # B0 chunks 0..2*F8_PAIRS-1 as fp8 DoubleRow pairs
U0 = (C0 - 2 * F8_PAIRS) // 2   # bf16 B0 units (2 chunks each): 4
U1 = C1 // 2              # 4
U2 = C2 // 2              # 4

SX = 4.0
SW = 256.0
INV_S = 2.0 ** -10
N_WARMUP = 10

BF16 = mybir.dt.bfloat16
FP8 = mybir.dt.float8e4
F32 = mybir.dt.float32


def build_nc():
    nc = bacc.Bacc("TRN2", target_bir_lowering=False, debug=False)

    x80_d = nc.dram_tensor("x80", [F8_PAIRS * P, 2, R], FP8, kind="ExternalInput").ap()
    xb0_d = nc.dram_tensor("xb0", [U0 * P, 2, R], BF16, kind="ExternalInput").ap()
    xb1_d = nc.dram_tensor("xb1", [U1 * P, 2, R], BF16, kind="ExternalInput").ap()
    xb2_d = nc.dram_tensor("xb2", [U2 * P, 2, R], BF16, kind="ExternalInput").ap()
    w80_d = nc.dram_tensor("w80", [F8_PAIRS * P, 2, K0], FP8, kind="ExternalInput").ap()
    wb0_d = nc.dram_tensor("wb0", [U0 * P, 2, K0], BF16, kind="ExternalInput").ap()
    wb1_d = nc.dram_tensor("wb1", [U1 * P, 2, K1], BF16, kind="ExternalInput").ap()
    wb2_d = nc.dram_tensor("wb2", [U2 * P, 2, K2], BF16, kind="ExternalInput").ap()
    out_d = nc.dram_tensor("out", [R, NOUT], BF16, kind="ExternalOutput").ap()

    rr = lambda d: d.rearrange("(u ci) s k -> ci u s k", ci=P)
    x80_r, xb0_r, xb1_r, xb2_r = rr(x80_d), rr(xb0_d), rr(xb1_d), rr(xb2_d)
    w80_r, wb0_r, wb1_r, wb2_r = rr(w80_d), rr(wb0_d), rr(wb1_d), rr(wb2_d)

    with tile.TileContext(nc) as tc:
        with (
            tc.tile_pool(name="x80p", bufs=F8_PAIRS) as x80p,
            tc.tile_pool(name="xb0p", bufs=U0) as xb0p,
            tc.tile_pool(name="xb1p", bufs=U1) as xb1p,
            tc.tile_pool(name="xb2p", bufs=U2) as xb2p,
            tc.tile_pool(name="w80p", bufs=F8_PAIRS) as w80p,
            tc.tile_pool(name="wb0p", bufs=U0) as wb0p,
            tc.tile_pool(name="wb1p", bufs=U1) as wb1p,
            tc.tile_pool(name="wb2p", bufs=U2) as wb2p,
            tc.tile_pool(name="cpool", bufs=1) as cpool,
            tc.tile_pool(name="opool", bufs=4) as opool,
            tc.tile_pool(name="psumA", bufs=BT, space="PSUM") as psumA_pool,
            tc.tile_pool(name="psumB", bufs=BT, space="PSUM") as psumB_pool,
        ):
            # scratch for PE warm-up: memset writes col 0 only; warm-ups
            # read a disjoint region so they carry no dependency.
            scratch = cpool.tile([P, P + K0], BF16, tag="scratch")
            nc.vector.memset(scratch[:, 0:1], 0.125)

            # unit schedule, phase 1 (co-major).  kinds: A=B0 bf16 unit,
            # F=B0 fp8 pair, S1/S2=B1/B2 unit.  B0 units are PE-rich per
            # DMA'd byte; S units are DMA-rich — interleave them.
            ph1 = []
            for i in range(3):
                ph1 += [("A", i), ("S1", i), ("F", i), ("S2", i)]
            ph1.append(("F", 3))
            # -> A0 S1_0 F0 S2_0 A1 S1_1 F1 S2_1 A2 S1_2 F2 S2_2 F3
            ph2 = [("A", 3), ("S1", 3), ("S2", 3)]

            pools = {"A": (xb0p, wb0p, xb0_r, wb0_r, BF16, R, K0, "b0"),
                     "F": (x80p, w80p, x80_r, w80_r, FP8, R, K0, "f0"),
                     "S1": (xb1p, wb1p, xb1_r, wb1_r, BF16, R, K1, "b1"),
                     "S2": (xb2p, wb2p, xb2_r, wb2_r, BF16, R, K2, "b2")}
            xt = {}
            wt = {}
            for kind, u in ph1 + ph2:
                xp, wp, xr, wr, dt, rdim, kdim, tag = pools[kind]
                xtt = xp.tile([P, 2, rdim], dt, tag="x" + tag)
                nc.sync.dma_start(xtt[:], xr[:, u, :, :])
                xt[(kind, u)] = xtt
                wtt = wp.tile([P, 2, kdim], dt, tag="w" + tag)
                nc.scalar.dma_start(wtt[:], wr[:, u, :, :])
                wt[(kind, u)] = wtt

            psA = [psumA_pool.tile([P, K0], F32, tag="psA", name=f"psA{b}")
                   for b in range(BT)]
            psB = [psumB_pool.tile([P, K1 + K2], F32, tag="psB", name=f"psB{b}")
                   for b in range(BT)]

            for i in range(N_WARMUP):
                nc.tensor.matmul(
                    psA[i % BT][:],
                    scratch[:, P:2 * P],
                    scratch[:, P:P + K0],
                    start=True, stop=True,
                )

            started = set()

            def unit_mms(kind, u, bt, stop=False):
                if kind == "F":
                    nc.tensor.matmul(
                        psA[bt][:],
                        xt[(kind, u)][:, :, ts(bt, P)],
                        wt[(kind, u)][:],
                        start=False, stop=False,
                        perf_mode=mybir.MatmulPerfMode.DoubleRow,
                    )
                    return
                # start=True clears has_written for the WHOLE bank, so only
                # the first matmul into each bank may set it; the other
                # region's first write overwrites (bit clear) correctly.
                if kind == "A":
                    dst = psA[bt][:]
                    bank = "A"
                elif kind == "S1":
                    dst = psB[bt][:, 0:K1]
                    bank = "B"
                else:
                    dst = psB[bt][:, K1:K1 + K2]
                    bank = "B"
                for s in range(2):
                    key = (bank, bt)
                    st = key not in started
                    started.add(key)
                    nc.tensor.matmul(
                        dst,
                        xt[(kind, u)][:, s, ts(bt, P)],
                        wt[(kind, u)][:, s, :],
                        start=st, stop=(stop and s == 1),
                    )

            # phase 1: co-major.  B0's psum group must start before any F
            # unit lands in it, so A0 leads the schedule.
            for kind, u in ph1:
                for bt in range(BT):
                    unit_mms(kind, u, bt)

            # phase 2: bt-major; epilogues overlap remaining matmuls
            for bt in range(BT):
                for kind, u in ph2:
                    unit_mms(kind, u, bt, stop=True)
                out_sb = opool.tile([P, NOUT], BF16, tag="osb")
                # rescale 2^-10: psA on ACT, psB on DVE (parallel engines)
                nc.scalar.activation(
                    out_sb[:, 0:K0], psA[bt][:],
                    mybir.ActivationFunctionType.Copy, scale=INV_S,
                )
                nc.vector.tensor_scalar_mul(
                    out_sb[:, K0:NOUT], psB[bt][:], INV_S,
                )
                nc.sync.dma_start(out_d[ts(bt, P), :], out_sb[:])

    nc.compile()
    return nc


def _nega_circ(v, m):
    i = np.arange(m)
    d = i[None, :] - i[:, None]
    return v[d % m] * np.where(d < 0, -1.0, 1.0)


def _circ(v, m):
    i = np.arange(m)
    return v[(i[None, :] - i[:, None]) % m]


def _pack_units(a, kdim):
    """[(nunits*2)*P, kdim] chunk-major -> [(nunits*P), 2, kdim]."""
    nu = a.shape[0] // (2 * P)
    return np.ascontiguousarray(
        a.reshape(nu, 2, P, kdim).transpose(0, 2, 1, 3).reshape(nu * P, 2, kdim)
    )


def prepare_in_maps(input, weight, bias):
    x = np.asarray(input, dtype=np.float64)
    w = np.asarray(weight, dtype=np.float64)

    v = w[(-np.arange(N)) % N]
    xp = x[:, :2048] + x[:, 2048:]
    xm = x[:, :2048] - x[:, 2048:]
    vp = v[:2048] + v[2048:]
    vm = v[:2048] - v[2048:]
    xpp = xp[:, :1024] + xp[:, 1024:]
    xpm = xp[:, :1024] - xp[:, 1024:]
    vpp = vp[:1024] + vp[1024:]
    vpm = vp[:1024] - vp[1024:]

    M0 = _nega_circ(vm, N0) * (0.5 * SW)
    M1 = _nega_circ(vpm, N1) * (0.25 * SW)
    M2 = _circ(vpp, N2) * (0.25 * SW)

    BF = ml_dtypes.bfloat16
    F8 = ml_dtypes.float8_e4m3fn
    nf8 = 2 * F8_PAIRS * P                      # contraction rows done in fp8

    xmT = np.ascontiguousarray(xm.T * SX)       # [2048, 1024]
    xpmT = np.ascontiguousarray(xpm.T * SX)     # [1024, 1024]
    xppT = np.ascontiguousarray(xpp.T * SX)

    in_maps = []
    for h in range(BH):
        rs = slice(h * R, (h + 1) * R)
        x80 = _pack_units(xmT[:nf8, rs].astype(np.float32).astype(F8), R)
        xb0 = _pack_units(xmT[nf8:, rs].astype(np.float32).astype(BF), R)
        xb1 = _pack_units(xpmT[:, rs].astype(np.float32).astype(BF), R)
        xb2 = _pack_units(xppT[:, rs].astype(np.float32).astype(BF), R)
        for q in range(CQ):
            w80 = _pack_units(
                M0[:nf8, q * K0:(q + 1) * K0].astype(np.float32).astype(F8), K0)
            wb0 = _pack_units(
                M0[nf8:, q * K0:(q + 1) * K0].astype(np.float32).astype(BF), K0)
            wb1 = _pack_units(
                M1[:, q * K1:(q + 1) * K1].astype(np.float32).astype(BF), K1)
            wb2 = _pack_units(
                M2[:, q * K2:(q + 1) * K2].astype(np.float32).astype(BF), K2)
            in_maps.append({"x80": x80, "xb0": xb0, "xb1": xb1, "xb2": xb2,
                            "w80": w80, "wb0": wb0, "wb1": wb1, "wb2": wb2})
    # core order: core = h*CQ + q
    return in_maps


def assemble(results, bias):
    """results: list of per-core {"out": [R, NOUT] bf16}; host recombine."""
    y0 = np.empty((BATCH, N0), np.float32)
    y1 = np.empty((BATCH, N1), np.float32)
    y2 = np.empty((BATCH, N2), np.float32)
    for h in range(BH):
        for q in range(CQ):
            o = results[h * CQ + q]["out"].astype(np.float32)
            rs = slice(h * R, (h + 1) * R)
            y0[rs, q * K0:(q + 1) * K0] = o[:, 0:K0]
            y1[rs, q * K1:(q + 1) * K1] = o[:, K0:K0 + K1]
            y2[rs, q * K2:(q + 1) * K2] = o[:, K0 + K1:NOUT]
    yp = np.concatenate([y2 + y1, y2 - y1], axis=1)
    out = np.concatenate([yp + y0, yp - y0], axis=1)
    return out + np.asarray(bias, np.float32)[None, :]


_NC_CACHE = None


def _get_nc():
    global _NC_CACHE
    if _NC_CACHE is None:
        _NC_CACHE = build_nc()
    return _NC_CACHE


def kernel(**inputs):
    nc = _get_nc()
    in_maps = prepare_in_maps(inputs["input"], inputs["weight"], inputs["bias"])
    res = run_bass_kernel_spmd(nc, in_maps, list(range(NCORES)))
    return assemble(res.results, inputs["bias"])


# revision 7
# speedup vs baseline: 1.7580x; 1.0598x over previous
"""Circulant matmul kernel for Trainium2 (8 NeuronCores, SPMD).

Problem: out = input @ K + bias, K[i, k] = weight[(i - k) mod 4096],
input [1024, 4096] f32, weight/bias [4096] f32.

Algorithm — 2-level real CRT splitting of the circulant (exact):
  out = x (circ-conv) v + bias, v[j] = w[(-j) mod n].
  R[X]/(X^4096-1) ~ R[X]/(X^2048-1) x R[X]/(X^2048+1); the cyclic factor
  splits once more.  Device work becomes three dense GEMMs
    B0: y0 = x_m  @ M0   (nega-circulant 2048, M0 = 1/2 * NC(v_m))
    B1: y1 = x_pm @ M1   (nega-circulant 1024, M1 = 1/4 * NC(v_pm))
    B2: y2 = x_pp @ M2   (circulant      1024, M2 = 1/4 *  C(v_pp))
  6.29M weight elems instead of 16.8M (37%); split butterflies and the
  final recombine are host-side pure adds (exact).

Sharding: 8 cores = 4 column shards x 2 batch halves.  B0 is 4-way
column-sharded (512 cols/core); B1/B2 are 2-way sharded with even q
cores taking B1 and odd q cores taking B2 (512 cols each, identical
shapes on every core -> SPMD).  Each core also only needs ONE of
x_pm/x_pp, cutting X traffic 25%.  No collectives; host recombines.

Precision (all blocks share scale product sx*sw = 1024; epilogue
multiplies by 2^-10):
  B0 chunks 0-3   fp8e4m3 DoubleRow pairs (sx=4,  sw=256)
  B0 chunks 4-11  fp8e3m4 (4-bit mantissa; bf16-speed matmuls but half
                  the DMA bytes; sx=2, sw=512)
  B0 chunks 12-15 and all of B1/B2: bf16 (sx=4, sw=256)
  Sim rel err 1.58e-2 vs the 2e-2 gate (HW runs ~+0.07e-2 over sim).

Device schedule (per core):
  - X units on the sync HWDGE ring, W units on the scalar ring; every
    unit is one [128, 2, 512] tile = 1 DMA; issue order = consumption
    order.  6 PE warm-up matmuls on a dep-free scratch tile lift the
    HAM clock gate exactly until the first real unit lands.
  - Phase 1 co-major (unit at a time across all 4 batch tiles); phase 2
    finishes batch tiles in turn so ACT/DVE epilogues and the output
    DMAs (sync ring) overlap remaining matmuls.
  - PSUM: psA[bt] (B0) + psB[bt] (B1|B2) = 8 banks.  start=True clears
    has_written for a whole bank, so only the first matmul into each
    bank sets it.
"""

import numpy as np
import ml_dtypes

import concourse.bass as bass
import concourse.mybir as mybir
import concourse.tile as tile
from concourse import bacc
from concourse.bass import ts
from concourse.bass_utils import run_bass_kernel_spmd

N = 4096
BATCH = 1024
NCORES = 8
CQ = 4                    # B0 column shards
BH = 2                    # batch halves
R = BATCH // BH           # 512 rows per core
P = 128
BT = R // P               # 4 batch tiles per core

N0 = 2048                 # B0 block size; B1/B2 are 1024
KC = 512                  # output cols per core per block shard
NOUT = 2 * KC             # 1024 out cols per core (B0 shard | Bx shard)

NU8, NU3, NUB, NUS = 2, 4, 2, 4   # units: e4m3 pairs, e3m4, bf16-B0, Bx

SX8, SW8 = 4.0, 256.0
SX3, SW3 = 2.0, 512.0
SXB, SWB = 4.0, 256.0
INV_S = 2.0 ** -10
N_WARMUP = 6

BF16 = mybir.dt.bfloat16
FP8E4 = mybir.dt.float8e4
FP8E3 = mybir.dt.float8e3
F32 = mybir.dt.float32


def build_nc():
    nc = bacc.Bacc("TRN2", target_bir_lowering=False, debug=False)

    def din(name, nu, dt):
        return nc.dram_tensor(name, [nu * P, 2, KC], dt, kind="ExternalInput").ap() \
                 .rearrange("(u ci) s k -> ci u s k", ci=P)

    x8_r = din("x8", NU8, FP8E4)
    x3_r = din("x3", NU3, FP8E3)
    xb_r = din("xb", NUB, BF16)
    xs_r = din("xs", NUS, BF16)
    w8_r = din("w8", NU8, FP8E4)
    w3_r = din("w3", NU3, FP8E3)
    wb_r = din("wb", NUB, BF16)
    ws_r = din("ws", NUS, BF16)
    out_d = nc.dram_tensor("out", [R, NOUT], BF16, kind="ExternalOutput").ap()

    with tile.TileContext(nc) as tc:
        with (
            tc.tile_pool(name="p2", bufs=2) as p2,
            tc.tile_pool(name="p4", bufs=4) as p4,
            tc.tile_pool(name="cpool", bufs=1) as cpool,
            tc.tile_pool(name="opool", bufs=4) as opool,
            tc.tile_pool(name="psumA", bufs=BT, space="PSUM") as psumA_pool,
            tc.tile_pool(name="psumB", bufs=BT, space="PSUM") as psumB_pool,
        ):
            scratch = cpool.tile([P, P + KC], BF16, tag="scratch")
            nc.vector.memset(scratch[:, 0:1], 0.125)

            # kinds: A=B0 bf16, E=B0 e3m4, F=B0 e4m3 DR pair, S=Bx bf16
            cfg = {"A": (p2, xb_r, wb_r, BF16, "xb", "wb"),
                   "E": (p4, x3_r, w3_r, FP8E3, "x3", "w3"),
                   "F": (p2, x8_r, w8_r, FP8E4, "x8", "w8"),
                   "S": (p4, xs_r, ws_r, BF16, "xs", "ws")}
            ph1 = [("A", 0), ("E", 0), ("S", 0), ("E", 1), ("F", 0),
                   ("S", 1), ("E", 2), ("S", 2), ("E", 3), ("F", 1)]
            ph2 = [("A", 1), ("S", 3)]

            xt, wt = {}, {}
            for kind, u in ph1 + ph2:
                pool, xr, wr, dt, xtag, wtag = cfg[kind]
                xtt = pool.tile([P, 2, R], dt, tag=xtag)
                nc.sync.dma_start(xtt[:], xr[:, u, :, :])
                xt[(kind, u)] = xtt
                wtt = pool.tile([P, 2, KC], dt, tag=wtag)
                nc.scalar.dma_start(wtt[:], wr[:, u, :, :])
                wt[(kind, u)] = wtt

            psA = [psumA_pool.tile([P, KC], F32, tag="psA", name=f"psA{b}")
                   for b in range(BT)]
            psB = [psumB_pool.tile([P, KC], F32, tag="psB", name=f"psB{b}")
                   for b in range(BT)]

            for i in range(N_WARMUP):
                nc.tensor.matmul(
                    psA[i % BT][:],
                    scratch[:, P:2 * P],
                    scratch[:, P:P + KC],
                    start=True, stop=True,
                )

            started = set()

            def unit_mms(kind, u, bt, stop=False):
                ps = psA[bt] if kind in ("A", "E", "F") else psB[bt]
                if kind == "F":
                    nc.tensor.matmul(
                        ps[:],
                        xt[(kind, u)][:, :, ts(bt, P)],
                        wt[(kind, u)][:],
                        start=False, stop=False,
                        perf_mode=mybir.MatmulPerfMode.DoubleRow,
                    )
                    return
                bank = "A" if kind in ("A", "E") else "B"
                for s in range(2):
                    key = (bank, bt)
                    st = key not in started
                    started.add(key)
                    nc.tensor.matmul(
                        ps[:],
                        xt[(kind, u)][:, s, ts(bt, P)],
                        wt[(kind, u)][:, s, :],
                        start=st, stop=(stop and s == 1),
                    )

            for kind, u in ph1:
                for bt in range(BT):
                    unit_mms(kind, u, bt)

            for bt in range(BT):
                for kind, u in ph2:
                    unit_mms(kind, u, bt, stop=True)
                out_sb = opool.tile([P, NOUT], BF16, tag="osb")
                nc.scalar.activation(
                    out_sb[:, 0:KC], psA[bt][:],
                    mybir.ActivationFunctionType.Copy, scale=INV_S,
                )
                nc.vector.tensor_scalar_mul(
                    out_sb[:, KC:NOUT], psB[bt][:], INV_S,
                )
                nc.sync.dma_start(out_d[ts(bt, P), :], out_sb[:])

    nc.compile()
    return nc


def _nega_circ(v, m):
    i = np.arange(m)
    d = i[None, :] - i[:, None]
    return v[d % m] * np.where(d < 0, -1.0, 1.0)


def _circ(v, m):
    i = np.arange(m)
    return v[(i[None, :] - i[:, None]) % m]


def _pack(a, dt, clip=False):
    """[(nu*2)*P, k] chunk-major f32 -> [(nu*P), 2, k] in dtype dt."""
    if clip:
        a = np.clip(a, -15.5, 15.5)
    k = a.shape[1]
    nu = a.shape[0] // (2 * P)
    return np.ascontiguousarray(
        a.reshape(nu, 2, P, k).transpose(0, 2, 1, 3).reshape(nu * P, 2, k)
    ).astype(dt)


def prepare_in_maps(input, weight, bias):
    x = np.asarray(input, dtype=np.float64)
    w = np.asarray(weight, dtype=np.float64)

    v = w[(-np.arange(N)) % N]
    xp = x[:, :2048] + x[:, 2048:]
    xm = x[:, :2048] - x[:, 2048:]
    vp = v[:2048] + v[2048:]
    vm = v[:2048] - v[2048:]
    xpp = xp[:, :1024] + xp[:, 1024:]
    xpm = xp[:, :1024] - xp[:, 1024:]
    vpp = vp[:1024] + vp[1024:]
    vpm = vp[:1024] - vp[1024:]

    M0 = _nega_circ(vm, N0) * 0.5
    M1 = _nega_circ(vpm, 1024) * 0.25
    M2 = _circ(vpp, 1024) * 0.25

    BF = ml_dtypes.bfloat16
    E4 = ml_dtypes.float8_e4m3fn
    E3 = ml_dtypes.float8_e3m4

    xmT = np.ascontiguousarray(xm.T).astype(np.float32)     # [2048, 1024]
    xpmT = np.ascontiguousarray(xpm.T).astype(np.float32)   # [1024, 1024]
    xppT = np.ascontiguousarray(xpp.T).astype(np.float32)
    M0 = M0.astype(np.float32)
    M1 = M1.astype(np.float32)
    M2 = M2.astype(np.float32)

    in_maps = []
    for h in range(BH):
        rs = slice(h * R, (h + 1) * R)
        x8 = _pack(xmT[0:512, rs] * SX8, E4)
        x3 = _pack(xmT[512:1536, rs] * SX3, E3, clip=True)
        xb = _pack(xmT[1536:2048, rs] * SXB, BF)
        xs_b1 = _pack(xpmT[:, rs] * SXB, BF)
        xs_b2 = _pack(xppT[:, rs] * SXB, BF)
        for q in range(CQ):
            cs = slice(q * KC, (q + 1) * KC)
            Mx = M1 if q % 2 == 0 else M2
            hs = slice((q // 2) * KC, (q // 2 + 1) * KC)
            in_maps.append({
                "x8": x8, "x3": x3, "xb": xb,
                "xs": xs_b1 if q % 2 == 0 else xs_b2,
                "w8": _pack(M0[0:512, cs] * SW8, E4),
                "w3": _pack(M0[512:1536, cs] * SW3, E3, clip=True),
                "wb": _pack(M0[1536:2048, cs] * SWB, BF),
                "ws": _pack(Mx[:, hs] * SWB, BF),
            })
    # core order: core = h*CQ + q
    return in_maps


def assemble(results, bias):
    """results: per-core {"out": [R, NOUT] bf16}; host butterflies + bias."""
    y0 = np.empty((BATCH, N0), np.float32)
    y1 = np.empty((BATCH, 1024), np.float32)
    y2 = np.empty((BATCH, 1024), np.float32)
    for h in range(BH):
        rs = slice(h * R, (h + 1) * R)
        for q in range(CQ):
            o = results[h * CQ + q]["out"].astype(np.float32)
            y0[rs, q * KC:(q + 1) * KC] = o[:, 0:KC]
            dst = y1 if q % 2 == 0 else y2
            dst[rs, (q // 2) * KC:(q // 2 + 1) * KC] = o[:, KC:NOUT]
    yp = np.concatenate([y2 + y1, y2 - y1], axis=1)
    out = np.concatenate([yp + y0, yp - y0], axis=1)
    return out + np.asarray(bias, np.float32)[None, :]


_NC_CACHE = None


def _get_nc():
    global _NC_CACHE
    if _NC_CACHE is None:
        _NC_CACHE = build_nc()
    return _NC_CACHE


def kernel(**inputs):
    nc = _get_nc()
    in_maps = prepare_in_maps(inputs["input"], inputs["weight"], inputs["bias"])
    res = run_bass_kernel_spmd(nc, in_maps, list(range(NCORES)))
    return assemble(res.results, inputs["bias"])


# revision 13
# speedup vs baseline: 1.7972x; 1.0222x over previous
"""Circulant matmul kernel for Trainium2 (8 NeuronCores, SPMD).

Problem: out = input @ K + bias, K[i, k] = weight[(i - k) mod 4096],
input [1024, 4096] f32, weight/bias [4096] f32.

Algorithm — 2-level real CRT splitting of the circulant (exact):
  out = x (circ-conv) v + bias, v[j] = w[(-j) mod n].
  R[X]/(X^4096-1) ~ R[X]/(X^2048-1) x R[X]/(X^2048+1); the cyclic factor
  splits once more.  Device work becomes three dense GEMMs
    B0: y0 = x_m  @ M0   (nega-circulant 2048, M0 = 1/2 * NC(v_m))
    B1: y1 = x_pm @ M1   (nega-circulant 1024, M1 = 1/4 * NC(v_pm))
    B2: y2 = x_pp @ M2   (circulant      1024, M2 = 1/4 *  C(v_pp))
  6.29M weight elems instead of 16.8M (37%); split butterflies and the
  final recombine are host-side pure adds (exact).

Sharding: 8 cores = 4 column shards x 2 batch halves.  B0 is 4-way
column-sharded (512 cols/core); B1/B2 are 2-way sharded with even q
cores taking B1 and odd q cores taking B2 (512 cols each, identical
shapes on every core -> SPMD).  Each core also only needs ONE of
x_pm/x_pp, cutting X traffic 25%.  No collectives; host recombines.

Precision (all blocks share scale product sx*sw = 1024; epilogue
multiplies by 2^-10):
  B0 chunks 0-3   fp8e4m3 DoubleRow pairs (sx=4,  sw=256)
  B0 chunks 4-11  fp8e3m4 (4-bit mantissa; bf16-speed matmuls but half
                  the DMA bytes; sx=2, sw=512)
  B0 chunks 12-15 and all of B1/B2: bf16 (sx=4, sw=256)
  Sim rel err 1.58e-2 vs the 2e-2 gate (HW runs ~+0.07e-2 over sim).

Device schedule (per core):
  - X units on the sync HWDGE ring, W units on the scalar ring; every
    unit is one [128, 2, 512] tile = 1 DMA; issue order = consumption
    order.  6 PE warm-up matmuls on a dep-free scratch tile lift the
    HAM clock gate exactly until the first real unit lands.
  - Phase 1 co-major (unit at a time across all 4 batch tiles); phase 2
    finishes batch tiles in turn so ACT/DVE epilogues and the output
    DMAs (sync ring) overlap remaining matmuls.
  - PSUM: psA[bt] (B0) + psB[bt] (B1|B2) = 8 banks.  start=True clears
    has_written for a whole bank, so only the first matmul into each
    bank sets it.
"""

import numpy as np
import ml_dtypes

import concourse.bass as bass
import concourse.mybir as mybir
import concourse.tile as tile
from concourse import bacc
from concourse.bass import ts
from concourse.bass_utils import run_bass_kernel_spmd

N = 4096
BATCH = 1024
NCORES = 8
CQ = 4                    # B0 column shards
BH = 2                    # batch halves
R = BATCH // BH           # 512 rows per core
P = 128
BT = R // P               # 4 batch tiles per core

N0 = 2048                 # B0 block size; B1/B2 are 1024
KC = 512                  # output cols per core per block shard
NOUT = 2 * KC             # 1024 out cols per core (B0 shard | Bx shard)

NU8, NU3, NUB, NUS = 1, 2, 1, 2   # 4-chunk units: e4m3 (2 DR pairs), e3m4, bf16-B0, Bx

SX8, SW8 = 4.0, 256.0
SX3, SW3 = 2.0, 512.0
SXB, SWB = 4.0, 256.0
INV_S = 2.0 ** -10
N_WARMUP = 6

BF16 = mybir.dt.bfloat16
FP8E4 = mybir.dt.float8e4
FP8E3 = mybir.dt.float8e3
F32 = mybir.dt.float32


def build_nc():
    nc = bacc.Bacc("TRN2", target_bir_lowering=False, debug=False)

    def din(name, nu, dt):
        return nc.dram_tensor(name, [nu * P, 4, KC], dt, kind="ExternalInput").ap() \
                 .rearrange("(u ci) s k -> ci u s k", ci=P)

    x8_r = din("x8", NU8, FP8E4)
    x3_r = din("x3", NU3, FP8E3)
    xb_r = din("xb", NUB, BF16)
    xs_r = din("xs", NUS, BF16)
    w8_r = din("w8", NU8, FP8E4)
    w3_r = din("w3", NU3, FP8E3)
    wb_r = din("wb", NUB, BF16)
    ws_r = din("ws", NUS, BF16)
    out_d = nc.dram_tensor("out", [R, NOUT], BF16, kind="ExternalOutput").ap()

    with tile.TileContext(nc) as tc:
        with (
            tc.tile_pool(name="p2", bufs=1) as p2,
            tc.tile_pool(name="p4", bufs=2) as p4,
            tc.tile_pool(name="cpool", bufs=1) as cpool,
            tc.tile_pool(name="opool", bufs=4) as opool,
            tc.tile_pool(name="psumA", bufs=BT, space="PSUM") as psumA_pool,
            tc.tile_pool(name="psumB", bufs=BT, space="PSUM") as psumB_pool,
        ):
            scratch = cpool.tile([P, P + KC], BF16, tag="scratch")
            nc.vector.memset(scratch[:, 0:1], 0.125)

            # kinds: A=B0 bf16, E=B0 e3m4, F=B0 e4m3 DR pairs, S=Bx bf16.
            # Each unit is one [128, 4, 512] tile = 4 chunks (F: 2 DR pairs).
            # E0 leads: half-size bytes, so the PE starts right as the
            # warm-ups finish.  The last slots of A0/S1 close the psum
            # banks bt-major in phase 2.
            cfg = {"A": (p2, xb_r, wb_r, BF16, "xb", "wb"),
                   "E": (p4, x3_r, w3_r, FP8E3, "x3", "w3"),
                   "F": (p2, x8_r, w8_r, FP8E4, "x8", "w8"),
                   "S": (p4, xs_r, ws_r, BF16, "xs", "ws")}
            # (kind, unit, slot_lo, slot_hi)
            ph1 = [("E", 0, 0, 4), ("S", 0, 0, 4), ("F", 0, 0, 4),
                   ("A", 0, 0, 2), ("E", 1, 0, 4), ("S", 1, 0, 2)]
            ph2 = [("A", 0, 2, 4), ("S", 1, 2, 4)]
            dma_order = [("E", 0), ("S", 0), ("F", 0), ("A", 0), ("E", 1),
                         ("S", 1)]

            xt, wt = {}, {}
            for kind, u in dma_order:
                pool, xr, wr, dt, xtag, wtag = cfg[kind]
                xtt = pool.tile([P, 4, R], dt, tag=xtag)
                nc.sync.dma_start(xtt[:], xr[:, u, :, :])
                xt[(kind, u)] = xtt
                wtt = pool.tile([P, 4, KC], dt, tag=wtag)
                nc.scalar.dma_start(wtt[:], wr[:, u, :, :])
                wt[(kind, u)] = wtt

            psA = [psumA_pool.tile([P, KC], F32, tag="psA", name=f"psA{b}")
                   for b in range(BT)]
            psB = [psumB_pool.tile([P, KC], F32, tag="psB", name=f"psB{b}")
                   for b in range(BT)]

            for i in range(N_WARMUP):
                nc.tensor.matmul(
                    psA[i % BT][:],
                    scratch[:, P:2 * P],
                    scratch[:, P:P + KC],
                    start=True, stop=True,
                )

            started = set()

            def unit_mms(kind, u, bt, s_lo, s_hi, stop=False):
                ps = psA[bt] if kind in ("A", "E", "F") else psB[bt]
                if kind == "F":
                    for pp in range(s_lo // 2, s_hi // 2):
                        nc.tensor.matmul(
                            ps[:],
                            xt[(kind, u)][:, 2 * pp:2 * pp + 2, ts(bt, P)],
                            wt[(kind, u)][:, 2 * pp:2 * pp + 2, :],
                            start=False, stop=False,
                            perf_mode=mybir.MatmulPerfMode.DoubleRow,
                        )
                    return
                bank = "A" if kind in ("A", "E") else "B"
                for s in range(s_lo, s_hi):
                    key = (bank, bt)
                    st = key not in started
                    started.add(key)
                    nc.tensor.matmul(
                        ps[:],
                        xt[(kind, u)][:, s, ts(bt, P)],
                        wt[(kind, u)][:, s, :],
                        start=st, stop=(stop and s == s_hi - 1),
                    )

            for kind, u, lo, hi in ph1:
                for bt in range(BT):
                    unit_mms(kind, u, bt, lo, hi)

            for bt in range(BT):
                for kind, u, lo, hi in ph2:
                    unit_mms(kind, u, bt, lo, hi, stop=True)
                out_sb = opool.tile([P, NOUT], BF16, tag="osb")
                nc.scalar.activation(
                    out_sb[:, 0:KC], psA[bt][:],
                    mybir.ActivationFunctionType.Copy, scale=INV_S,
                )
                nc.vector.tensor_scalar_mul(
                    out_sb[:, KC:NOUT], psB[bt][:], INV_S,
                )
                nc.sync.dma_start(out_d[ts(bt, P), :], out_sb[:])

    nc.compile()
    return nc


def _nega_circ(v, m):
    i = np.arange(m)
    d = i[None, :] - i[:, None]
    return v[d % m] * np.where(d < 0, -1.0, 1.0)


def _circ(v, m):
    i = np.arange(m)
    return v[(i[None, :] - i[:, None]) % m]


def _pack(a, dt, clip=False):
    """[(nu*4)*P, k] chunk-major f32 -> [(nu*P), 4, k] in dtype dt."""
    if clip:
        a = np.clip(a, -15.5, 15.5)
    k = a.shape[1]
    nu = a.shape[0] // (4 * P)
    return np.ascontiguousarray(
        a.reshape(nu, 4, P, k).transpose(0, 2, 1, 3).reshape(nu * P, 4, k)
    ).astype(dt)


def prepare_in_maps(input, weight, bias):
    x = np.asarray(input, dtype=np.float64)
    w = np.asarray(weight, dtype=np.float64)

    v = w[(-np.arange(N)) % N]
    xp = x[:, :2048] + x[:, 2048:]
    xm = x[:, :2048] - x[:, 2048:]
    vp = v[:2048] + v[2048:]
    vm = v[:2048] - v[2048:]
    xpp = xp[:, :1024] + xp[:, 1024:]
    xpm = xp[:, :1024] - xp[:, 1024:]
    vpp = vp[:1024] + vp[1024:]
    vpm = vp[:1024] - vp[1024:]

    M0 = _nega_circ(vm, N0) * 0.5
    M1 = _nega_circ(vpm, 1024) * 0.25
    M2 = _circ(vpp, 1024) * 0.25

    BF = ml_dtypes.bfloat16
    E4 = ml_dtypes.float8_e4m3fn
    E3 = ml_dtypes.float8_e3m4

    xmT = np.ascontiguousarray(xm.T).astype(np.float32)     # [2048, 1024]
    xpmT = np.ascontiguousarray(xpm.T).astype(np.float32)   # [1024, 1024]
    xppT = np.ascontiguousarray(xpp.T).astype(np.float32)
    M0 = M0.astype(np.float32)
    M1 = M1.astype(np.float32)
    M2 = M2.astype(np.float32)

    in_maps = []
    for h in range(BH):
        rs = slice(h * R, (h + 1) * R)
        x8 = _pack(xmT[0:512, rs] * SX8, E4)
        x3 = _pack(xmT[512:1536, rs] * SX3, E3, clip=True)
        xb = _pack(xmT[1536:2048, rs] * SXB, BF)
        xs_b1 = _pack(xpmT[:, rs] * SXB, BF)
        xs_b2 = _pack(xppT[:, rs] * SXB, BF)
        for q in range(CQ):
            cs = slice(q * KC, (q + 1) * KC)
            Mx = M1 if q % 2 == 0 else M2
            hs = slice((q // 2) * KC, (q // 2 + 1) * KC)
            in_maps.append({
                "x8": x8, "x3": x3, "xb": xb,
                "xs": xs_b1 if q % 2 == 0 else xs_b2,
                "w8": _pack(M0[0:512, cs] * SW8, E4),
                "w3": _pack(M0[512:1536, cs] * SW3, E3, clip=True),
                "wb": _pack(M0[1536:2048, cs] * SWB, BF),
                "ws": _pack(Mx[:, hs] * SWB, BF),
            })
    # core order: core = h*CQ + q
    return in_maps


def assemble(results, bias):
    """results: per-core {"out": [R, NOUT] bf16}; host butterflies + bias."""
    y0 = np.empty((BATCH, N0), np.float32)
    y1 = np.empty((BATCH, 1024), np.float32)
    y2 = np.empty((BATCH, 1024), np.float32)
    for h in range(BH):
        rs = slice(h * R, (h + 1) * R)
        for q in range(CQ):
            o = results[h * CQ + q]["out"].astype(np.float32)
            y0[rs, q * KC:(q + 1) * KC] = o[:, 0:KC]
            dst = y1 if q % 2 == 0 else y2
            dst[rs, (q // 2) * KC:(q // 2 + 1) * KC] = o[:, KC:NOUT]
    yp = np.concatenate([y2 + y1, y2 - y1], axis=1)
    out = np.concatenate([yp + y0, yp - y0], axis=1)
    return out + np.asarray(bias, np.float32)[None, :]


_NC_CACHE = None


def _get_nc():
    global _NC_CACHE
    if _NC_CACHE is None:
        _NC_CACHE = build_nc()
    return _NC_CACHE


def kernel(**inputs):
    nc = _get_nc()
    in_maps = prepare_in_maps(inputs["input"], inputs["weight"], inputs["bias"])
    res = run_bass_kernel_spmd(nc, in_maps, list(range(NCORES)))
    return assemble(res.results, inputs["bias"])
